# revision 18
# baseline (speedup 1.0000x reference)
"""TRN2 Bass kernel for nn_DecoderLayer_47175920779446.

Full decoder layer: qkv (mul-bias) -> 16-head attention -> +res -> LN ->
FFN(relu, mul-bias) -> +res -> LN, on x[2, 2048, 1024] fp32.

Sharding (8 cores): attention is sharded by (batch, 4 heads): core c handles
batch c//4, heads 4*(c%4)..4*(c%4)+3 over all 2048 tokens of its batch.
Attention output resharding uses FOUR per-query-group AllToAlls (fired as
each group's heads complete, so they overlap stage_b): FFN tokens are
assigned uniformly -- core d owns, for each group g, batch-0 tokens
[512g+64d, +64) and batch-1 tokens [512g+64d, +64). LN1/FFN/LN2 then run
token-sharded (512 tokens per core) with replicated weights, pipelined per
128-token chunk behind the collectives.

Precision: scores need ~fp32 accuracy (std ~256 feeding exp): q,k chain runs
fp32r (11-bit mantissa) projections, then an exact bf16 hi/lo split with a
2-matmul scheme: S = qh*kh + m_hat (main, K=65 with a fused bias row) plus
[qh;ql]*[kl;kh] (cross, K=128). V/P/FFN run bf16; residuals/LN run fp32.
"""
import contextlib
import numpy as np
import ml_dtypes

import concourse.bass as bass
import concourse.tile as tile
from concourse import bacc, mybir
from concourse.bass_utils import run_bass_kernel_spmd
from concourse.bass_interp import get_hw_module
from concourse.masks import make_identity

H, NH, HD, FF = 1024, 16, 64, 4096
B, T = 2, 2048
EPS = 1e-6
NCORES = 8
HPC = NH // 4          # 4 heads per core
TOK = (B * T) // NCORES  # 512 tokens per core
NKC = T // 128         # 16 key chunks
NG = T // 512          # 4 query groups
KCH = H // 128         # 8 contraction chunks for qkv
f32, f32r, bf16 = mybir.dt.float32, mybir.dt.float32r, mybir.dt.bfloat16
AF = mybir.ActivationFunctionType
ALU = mybir.AluOpType


def _round_mant(x, bits=11):
    xi = np.ascontiguousarray(x, np.float32).view(np.int32)
    shift = 23 - bits
    bias = (1 << (shift - 1)) - 1 + ((xi >> shift) & 1)
    xi = (xi + bias) & ~((1 << shift) - 1)
    return xi.view(np.float32)


def _build_program(sim_single=False):
    nc = bacc.Bacc("TRN2", target_bir_lowering=False, debug=False,
                   num_devices=1 if sim_single else NCORES)
    ap = {}
    ap["xT"] = nc.dram_tensor("xT", [H, T], f32r, kind="ExternalInput").ap()
    ap["xres"] = nc.dram_tensor("xres", [TOK, H], f32, kind="ExternalInput").ap()
    for w in ("wq", "wk", "wv"):
        ap[w] = nc.dram_tensor(w, [H, 4 * HD], f32r, kind="ExternalInput").ap()
    ap["w1"] = nc.dram_tensor("w1", [H, FF], bf16, kind="ExternalInput").ap()
    ap["w2"] = nc.dram_tensor("w2", [FF, H], bf16, kind="ExternalInput").ap()
    ap["lnw"] = nc.dram_tensor("lnw", [4, H], f32, kind="ExternalInput").ap()
    out_ap = nc.dram_tensor("out", [TOK, H], f32, kind="ExternalOutput").ap()

    with tile.TileContext(nc) as tc:
        ctx = contextlib.ExitStack()
        with ctx:
            const = ctx.enter_context(tc.tile_pool(name="const", bufs=1))
            dram = ctx.enter_context(tc.tile_pool(name="dram", bufs=1, space="DRAM"))

            ident = const.tile([128, 128], f32)
            make_identity(nc, ident[:])
            identB = const.tile([128, 128], bf16)
            make_identity(nc, identB[:])

            # per-group collective buffers: [dst core, 64 tok, 4 heads * 64]
            a2a_in = [dram.tile([NCORES, 64, 4 * HD], f32, name=f"a2ai{g}")
                      for g in range(NG)]
            a2a_out = [dram.tile([NCORES, 64, 4 * HD], f32, name=f"a2ao{g}")
                       for g in range(NG)]

            # ---------------- attention scope ----------------
            actx = contextlib.ExitStack()
            with actx:
                qk = actx.enter_context(tc.tile_pool(name="qk", bufs=1))
                sb = actx.enter_context(tc.tile_pool(name="sb", bufs=3))
                small = actx.enter_context(tc.tile_pool(name="small", bufs=4))
                psn = actx.enter_context(
                    tc.tile_pool(name="psn", bufs=2, space="PSUM"))
                pss = actx.enter_context(
                    tc.tile_pool(name="pss", bufs=2, space="PSUM"))
                pso = actx.enter_context(
                    tc.tile_pool(name="pso", bufs=1, space="PSUM"))
                psm = actx.enter_context(
                    tc.tile_pool(name="psm", bufs=1, space="PSUM"))

                # per-head score operands
                til_q, til_k, cr_q, cr_k = {}, {}, {}, {}
                for h in range(HPC):
                    til_q[h] = qk.tile([65, T], bf16, name=f"til_q{h}", tag="tq", bufs=HPC)
                    til_k[h] = qk.tile([65, T], bf16, name=f"til_k{h}", tag="tk", bufs=HPC)
                    cr_q[h] = qk.tile([128, T], bf16, name=f"cr_q{h}", tag="cq", bufs=HPC)
                    cr_k[h] = qk.tile([128, T], bf16, name=f"cr_k{h}", tag="ck", bufs=HPC)
                    nc.gpsimd.memset(til_k[h][64:65, :], 1.0)
                vn = []
                for kc in range(NKC):
                    v = qk.tile([128, HPC, 65], bf16, name=f"vn{kc}", tag="vn", bufs=NKC)
                    nc.gpsimd.memset(v[:, :, 64:65], 1.0)
                    vn.append(v)

                # ---- QKV projection (own scope: weights + xT staging free
                # early so the w1 prefetch can reuse the SBUF) ----
                pctx = contextlib.ExitStack()
                with pctx:
                    wpool = pctx.enter_context(tc.tile_pool(name="wpool", bufs=1))
                    xgp = pctx.enter_context(tc.tile_pool(name="xgp", bufs=4))

                    w_sb = {}
                    for w in ("wq", "wk", "wv"):
                        w_sb[w] = wpool.tile([128, KCH, 4 * HD], f32r, name=f"sb_{w}")
                    xgs = [xgp.tile([128, KCH, 512], f32r, name=f"xg{g}",
                                    tag="xg", bufs=4) for g in range(NG)]
                    # chunked loads, K-path first so wk matmuls start ~1us in
                    wrr = {w: ap[w].rearrange("(a p) c -> p a c", p=128)
                           for w in ("wq", "wk", "wv")}
                    xrr = ap["xT"].rearrange("(a p) t -> p a t", p=128)
                    for a in range(KCH):
                        nc.sync.dma_start(w_sb["wk"][:, a, :], wrr["wk"][:, a, :])
                        nc.sync.dma_start(
                            xgs[0][:, a, :], xrr[:, a, 0:512])
                    for g in range(1, NG):
                        for a in range(KCH):
                            nc.sync.dma_start(
                                xgs[g][:, a, :], xrr[:, a, 512 * g:512 * (g + 1)])
                    for w in ("wq", "wv"):
                        for a in range(KCH):
                            nc.sync.dma_start(w_sb[w][:, a, :], wrr[w][:, a, :])

                    def proj_pass(name, til, cr, g):
                        gsl = slice(512 * g, 512 * (g + 1))
                        for hp in range(2):  # head pairs
                            p = pss.tile([128, 512], f32, tag="st", name="pqk")
                            for a in range(KCH):
                                nc.tensor.matmul(
                                    p[:], w_sb[name][:, a, 128 * hp:128 * (hp + 1)],
                                    xgs[g][:, a, :], start=(a == 0), stop=(a == KCH - 1))
                            for hl in range(2):
                                h = 2 * hp + hl
                                rows = slice(64 * hl, 64 * (hl + 1))
                                nc.scalar.activation(til[h][0:64, gsl], p[rows, :], AF.Copy)
                                if name == "wq":
                                    hi_rows, lo_rows = slice(0, 64), slice(64, 128)
                                else:
                                    hi_rows, lo_rows = slice(64, 128), slice(0, 64)
                                nc.sync.dma_start(cr[h][hi_rows, gsl], til[h][0:64, gsl])
                                nc.vector.scalar_tensor_tensor(
                                    out=cr[h][lo_rows, gsl], in0=p[rows, :], scalar=1.0,
                                    in1=til[h][0:64, gsl], op0=ALU.mult, op1=ALU.subtract)

                    for g in range(NG):
                        proj_pass("wk", til_k, cr_k, g)
                    for g in range(NG):
                        proj_pass("wq", til_q, cr_q, g)
                        for tt in range(4):  # V natural per token tile
                            kc = 4 * g + tt
                            p = pss.tile([128, 4 * HD], f32, tag="st", name="pv")
                            for a in range(KCH):
                                nc.tensor.matmul(
                                    p[:], xgs[g][:, a, 128 * tt:128 * (tt + 1)],
                                    w_sb["wv"][:, a, :], start=(a == 0), stop=(a == KCH - 1))
                            nc.scalar.activation(
                                vn[kc][:, :, 0:64],
                                p[:].rearrange("p (h d) -> p h d", h=HPC), AF.Copy)
                # pctx closed: reserve right-side SBUF for the FFN1 weights
                # (streamed in during attention; the left side stays clear of
                # the attention tiles, so no WAR serialization) and for the
                # LN1-side tiles that run inside the attention loop.
                w1p = ctx.enter_context(
                    tc.tile_pool(name="w1p", bufs=1, side="right"))
                lnp = ctx.enter_context(
                    tc.tile_pool(name="lnp", bufs=1, side="right"))
                w1full = w1p.tile([128, KCH, FF], bf16, name="w1full")
                w1rr = ap["w1"].rearrange("(a p) f -> p a f", p=128)

                def prefetch_chunk(i):
                    # w1 by f-blocks, in FFN1 consumption order
                    fsl = slice(512 * i, 512 * (i + 1))
                    nc.sync.dma_start(w1full[:, :, fsl], w1rr[:, :, fsl])

                lnbc1 = {}
                for i, nm in enumerate(("g1", "b1")):
                    lnbc1[nm] = lnp.tile([128, H], f32, name=f"ln_{nm}",
                                         tag="lnbc1", bufs=2)
                    nc.sync.dma_start(
                        lnbc1[nm][:], ap["lnw"][i, :].partition_broadcast(128))

                def layer_norm_to(dst, src, g_bc, b_bc, work, pool):
                    """dst = gamma*(src-mean)/(std_unbiased+EPS)+beta, [128,H]."""
                    stats = pool.tile([128, 2, 6], f32, tag="stats", name="stats",
                                      bufs=2)
                    for hf in range(2):
                        nc.vector.bn_stats(stats[:, hf, :],
                                           src[:, 512 * hf:512 * (hf + 1)])
                    mv = pool.tile([128, 2], f32, tag="mv", name="mv", bufs=2)
                    nc.vector.bn_aggr(mv[:], stats[:])
                    sd = pool.tile([128, 1], f32, tag="sd", name="sd", bufs=2)
                    nc.scalar.activation(sd[:], mv[:, 1:2], AF.Sqrt,
                                         scale=float(H) / (H - 1))
                    nc.vector.tensor_scalar_add(sd[:], sd[:], EPS)
                    rs = pool.tile([128, 1], f32, tag="rs", name="rs", bufs=2)
                    nc.vector.reciprocal(rs[:], sd[:])
                    nc.vector.tensor_scalar(out=work[:], in0=src[:],
                                            scalar1=mv[:, 0:1], scalar2=rs[:],
                                            op0=ALU.subtract, op1=ALU.mult)
                    nc.vector.tensor_mul(work[:], work[:], g_bc[:])
                    nc.vector.tensor_add(dst[:], work[:], b_bc[:])

                out1c = {}

                def ln1(g):
                    # gather this core's 128 tokens of chunk g: rows 0:64 from
                    # batch-0 sources (0..3), 64:128 from batch-1 (4..7)
                    at = lnp.tile([128, H], f32, tag="ta", name="at", bufs=2)
                    for s in range(4):
                        csl = slice(256 * s, 256 * (s + 1))
                        nc.sync.dma_start(at[0:64, csl], a2a_out[g][s, :, :])
                        nc.sync.dma_start(at[64:128, csl], a2a_out[g][s + 4, :, :])
                    xt = lnp.tile([128, H], f32, tag="tc", name="xt", bufs=1)
                    nc.sync.dma_start(xt[:], ap["xres"][128 * g:128 * (g + 1), :])
                    nc.vector.tensor_add(at[:], at[:], xt[:])
                    wk_ = lnp.tile([128, H], f32, tag="td", name="wk_", bufs=1)
                    o1 = lnp.tile([128, H], bf16, name="out1c", tag="o1", bufs=4)
                    layer_norm_to(o1[:], at, lnbc1["g1"], lnbc1["b1"], wk_, lnp)
                    out1c[g] = o1

                # ---- attention, software-pipelined over (group, head) units
                # g-major so each group's AllToAll fires 1/4 of the way in.
                units = [(h, g) for g in range(NG) for h in range(HPC)]

                def stage_a1(h, g):
                    # natural-S matmuls + DVE max reduces (no PE dependency on DVE)
                    mstage = small.tile([128, 4], bf16, tag="mstage", name="mstage", bufs=2)
                    for qt in range(4):
                        qsl = slice(512 * g + 128 * qt, 512 * g + 128 * (qt + 1))
                        negmax = []
                        for half in range(2):
                            sn = psn.tile([128, 1024], f32, name="sn")
                            for j in range(2):
                                ks = slice(1024 * half + 512 * j,
                                           1024 * half + 512 * (j + 1))
                                nc.tensor.matmul(
                                    sn[:, 512 * j:512 * (j + 1)],
                                    til_q[h][0:64, qsl], til_k[h][0:64, ks],
                                    start=True, stop=True)
                            nm = small.tile([128, 1], bf16, tag="nm", name="nm")
                            nc.vector.tensor_reduce(
                                nm[:], sn[:], axis=mybir.AxisListType.X,
                                op=ALU.max, negate=True)
                            negmax.append(nm)
                        nc.vector.tensor_tensor(
                            mstage[:, qt:qt + 1], negmax[0][:], negmax[1][:], ALU.min)
                    return mstage

                def stage_a2(h, g, mstage):
                    # emitted a period later so the PE transpose never waits on DVE
                    for qt in range(4):
                        qsl = slice(512 * g + 128 * qt, 512 * g + 128 * (qt + 1))
                        mt = psm.tile([1, 128], bf16, tag="mt", name="mt")
                        nc.tensor.transpose(mt[:], mstage[:, qt:qt + 1], identB[:])
                        nc.vector.tensor_copy(til_q[h][64:65, qsl], mt[:])

                def stage_b(h, g):
                    gsl = slice(512 * g, 512 * (g + 1))
                    o_acc = pso.tile([65, 512], f32, name="o_acc")
                    pts = {}
                    PVLAG = 2

                    def pv(kc):
                        nc.tensor.matmul(o_acc[:], vn[kc][:, h, :], pts.pop(kc)[:],
                                         start=(kc == 0), stop=(kc == NKC - 1))

                    for kc in range(NKC):
                        ksl = slice(128 * kc, 128 * (kc + 1))
                        st = pss.tile([128, 512], f32, tag="st", name="st")
                        nc.tensor.matmul(st[:], til_k[h][0:65, ksl],
                                         til_q[h][0:65, gsl], start=True, stop=False)
                        nc.tensor.matmul(st[:], cr_k[h][:, ksl],
                                         cr_q[h][:, gsl], start=False, stop=True)
                        pt = sb.tile([128, 512], bf16, tag="pt", name="pt", bufs=6)
                        nc.scalar.activation(pt[:], st[:], AF.Exp)
                        pts[kc] = pt
                        if kc >= PVLAG:
                            pv(kc - PVLAG)
                    for kc in range(NKC - PVLAG, NKC):
                        pv(kc)
                    ot = sb.tile([65, 512], f32, tag="ot", name="ot", bufs=2)
                    nc.scalar.activation(ot[:], o_acc[:], AF.Copy)
                    # transpose to natural, scale by 1/denom, ship to a2a_in:
                    # rows 0:64 of token tile tt go to dst core 2tt, 64:128 to
                    # 2tt+1 (uniform token ownership, no duplication).
                    for tt in range(4):
                        op_ = psm.tile([128, 65], f32, tag="mt", name="opt")
                        nc.tensor.transpose(
                            op_[:], ot[0:65, 128 * tt:128 * (tt + 1)],
                            ident[0:65, 0:65])
                        rc = small.tile([128, 1], f32, tag="rc", name="rc")
                        nc.vector.reciprocal(rc[:], op_[:, 64:65])
                        ob = sb.tile([128, HD], f32, tag="ob", name="ob", bufs=4)
                        nc.vector.tensor_scalar_mul(ob[:], op_[:, 0:64], rc[:])
                        nc.sync.dma_start(
                            a2a_in[g][2 * tt, :, 64 * h:64 * (h + 1)], ob[0:64, :])
                        nc.sync.dma_start(
                            a2a_in[g][2 * tt + 1, :, 64 * h:64 * (h + 1)], ob[64:128, :])

                def fire_collective(g):
                    if sim_single:
                        # timing stand-in for the 512KB per-group AllToAll
                        nc.sync.dma_start(a2a_out[g][:], a2a_in[g][:])
                    else:
                        nc.gpsimd.collective_compute(
                            "AllToAll", ALU.bypass,
                            replica_groups=[list(range(NCORES))],
                            ins=[a2a_in[g].opt()], outs=[a2a_out[g].opt()])

                LOOKAHEAD = 3
                mstages, done_a2 = {}, set()
                for k in range(min(LOOKAHEAD, len(units))):
                    mstages[k] = stage_a1(*units[k])
                    stage_a2(*units[k], mstages.pop(k))
                    done_a2.add(k)
                for i, (h, g) in enumerate(units):
                    j = i + LOOKAHEAD
                    if j < len(units):
                        mstages[j] = stage_a1(*units[j])
                    j2 = i + LOOKAHEAD - 1
                    if j2 < len(units) and j2 not in done_a2:
                        stage_a2(*units[j2], mstages.pop(j2))
                        done_a2.add(j2)
                    stage_b(h, g)
                    if 1 <= i <= KCH:
                        prefetch_chunk(i - 1)
                    if h == HPC - 1:
                        fire_collective(g)
                        ln1(g)

            # ---------------- FFN / LN tail: pipelined per 128-token chunk --
            # (LN1 for every chunk already ran inside the attention loop.)
            fctx = contextlib.ExitStack()
            with fctx:
                fsb = fctx.enter_context(tc.tile_pool(name="fsb", bufs=1))
                o1p = fctx.enter_context(tc.tile_pool(name="o1p", bufs=1))
                w2p = fctx.enter_context(tc.tile_pool(name="w2p", bufs=1))
                fsm = fctx.enter_context(tc.tile_pool(name="fsm", bufs=2))
                psf = fctx.enter_context(
                    tc.tile_pool(name="psf", bufs=2, space="PSUM"))
                psg = fctx.enter_context(
                    tc.tile_pool(name="psg", bufs=2, space="PSUM"))

                lnbc2 = {}
                for i, nm in enumerate(("g2", "b2")):
                    lnbc2[nm] = o1p.tile([128, H], f32, name=f"ln_{nm}",
                                         tag="lnbc2", bufs=2)
                    nc.sync.dma_start(
                        lnbc2[nm][:], ap["lnw"][2 + i, :].partition_broadcast(128))

                # w2 lands in SBUF freed by the attention tiles; FFN2 consumes
                # ft ascending, so chunk the load in that order.
                w2full = w2p.tile([128, FF // 128, H], bf16, name="w2full")
                w2rr = ap["w2"].rearrange("(a p) o -> p a o", p=128)
                for c in range(8):
                    nc.gpsimd.dma_start(
                        w2full[:, 4 * c:4 * (c + 1), :],
                        w2rr[:, 4 * c:4 * (c + 1), :])

                out1T, htc = {}, {}

                def transp1(g):
                    o1T = o1p.tile([128, KCH, 128], bf16, name="out1T",
                                   tag="o1T", bufs=2)
                    for a in range(KCH):
                        tp = psg.tile([128, 128], bf16, tag="tp", name="tp", bufs=2)
                        nc.tensor.transpose(
                            tp[:], out1c[g][:, 128 * a:128 * (a + 1)], identB[:])
                        nc.scalar.activation(o1T[:, a, :], tp[:], AF.Copy)
                    out1T[g] = o1T

                def ffn1(g):
                    ht = o1p.tile([128, FF // 128, 128], bf16, name="htc",
                                  tag="ht", bufs=1)
                    for fb in range(KCH):  # 8 blocks of 512 ff rows
                        hp_ = psf.tile([128, 512], f32, tag="hp", name="hp", bufs=3)
                        for fq in range(4):
                            for a in range(KCH):
                                nc.tensor.matmul(
                                    hp_[:, 128 * fq:128 * (fq + 1)],
                                    w1full[:, a, 512 * fb + 128 * fq:512 * fb + 128 * (fq + 1)],
                                    out1T[g][:, a, :], start=(a == 0), stop=(a == KCH - 1))
                        nc.scalar.activation(
                            ht[:, 4 * fb:4 * (fb + 1), :],
                            hp_[:].rearrange("p (q t) -> p q t", q=4), AF.Relu)
                    htc[g] = ht

                def ffn2_ln2(g):
                    h2 = fsb.tile([128, H], f32, tag="ta", name="h2", bufs=2)
                    for oc in range(2):
                        acc = psf.tile([128, 512], f32, tag="o2", name="o2acc", bufs=2)
                        for ft in range(FF // 128):
                            nc.tensor.matmul(
                                acc[:], htc[g][:, ft, :],
                                w2full[:, ft, 512 * oc:512 * (oc + 1)],
                                start=(ft == 0), stop=(ft == FF // 128 - 1))
                        nc.vector.tensor_add(
                            h2[:, 512 * oc:512 * (oc + 1)],
                            out1c[g][:, 512 * oc:512 * (oc + 1)], acc[:])
                    fin = fsb.tile([128, H], f32, tag="tb", name="fin", bufs=2)
                    wk2 = fsb.tile([128, H], f32, tag="tc", name="wk2", bufs=2)
                    layer_norm_to(fin, h2, lnbc2["g2"], lnbc2["b2"], wk2, fsm)
                    nc.sync.dma_start(out_ap[128 * g:128 * (g + 1), :], fin[:])

                for g in range(NG):
                    transp1(g)
                    ffn1(g)
                    ffn2_ln2(g)

    nc.compile()
    if not sim_single:
        nc.m = get_hw_module(nc.m)
    return nc


_NC_CACHE = {}


def _get_program():
    if "nc" not in _NC_CACHE:
        _NC_CACHE["nc"] = _build_program()
    return _NC_CACHE["nc"]


def _prep_inputs(x, Wqkv, bqkv, W1, b1, W2, b2, gamma1, beta1, gamma2, beta2):
    """Host-side slicing/folding into per-core in_maps."""
    x = np.asarray(x, np.float32)
    Wqkv = np.asarray(Wqkv, np.float32)
    bqkv = np.asarray(bqkv, np.float32)
    d = np.arange(HD)
    hh = np.arange(NH)
    # qkv reshape in reference: [B,T,HD,3,NH] -> col = d*48 + k*16 + h
    cols = d[:, None, None] * (3 * NH) + np.arange(3)[None, :, None] * NH \
        + hh[None, None, :]
    Wq = Wqkv[:, cols[:, 0, :]] * (bqkv[cols[:, 0, :]] / np.sqrt(H))[None]
    Wk = Wqkv[:, cols[:, 1, :]] * bqkv[cols[:, 1, :]][None]
    Wv = Wqkv[:, cols[:, 2, :]] * bqkv[cols[:, 2, :]][None]
    # -> [H, HD, NH]; per-core head-major layout [H, 4*HD] (head-local major)
    Wq = np.transpose(Wq, (0, 2, 1))  # [H, NH, HD]
    Wk = np.transpose(Wk, (0, 2, 1))
    Wv = np.transpose(Wv, (0, 2, 1))
    W1e = (np.asarray(W1, np.float32) * np.asarray(b1, np.float32)[None]) \
        .astype(ml_dtypes.bfloat16)
    W2e = (np.asarray(W2, np.float32) * np.asarray(b2, np.float32)[None]) \
        .astype(ml_dtypes.bfloat16)
    lnw = np.stack([gamma1, beta1, gamma2, beta2]).astype(np.float32)
    xT = [_round_mant(np.ascontiguousarray(x[b].T)) for b in range(B)]
    in_maps = []
    for c in range(NCORES):
        b, grp = c // 4, c % 4
        heads = slice(4 * grp, 4 * grp + 4)
        # xres rows for core c: per group g, 64 batch-0 then 64 batch-1 tokens
        xres = np.concatenate([
            np.concatenate([x[0, 512 * g + 64 * c:512 * g + 64 * c + 64, :],
                            x[1, 512 * g + 64 * c:512 * g + 64 * c + 64, :]])
            for g in range(NG)])
        in_maps.append({
            "xT": xT[b],
            "xres": np.ascontiguousarray(xres),
            "wq": _round_mant(Wq[:, heads, :].reshape(H, 4 * HD)),
            "wk": _round_mant(Wk[:, heads, :].reshape(H, 4 * HD)),
            "wv": _round_mant(Wv[:, heads, :].reshape(H, 4 * HD)),
            "w1": W1e, "w2": W2e, "lnw": lnw,
        })
    return in_maps


def kernel(x, Wqkv, bqkv, W1, b1, W2, b2, gamma1, beta1, gamma2, beta2,
           _trace=False):
    nc = _get_program()
    in_maps = _prep_inputs(x, Wqkv, bqkv, W1, b1, W2, b2,
                           gamma1, beta1, gamma2, beta2)
    res = run_bass_kernel_spmd(nc, in_maps, core_ids=list(range(NCORES)),
                               trace=_trace)
    out = np.empty((B, T, H), np.float32)
    for c in range(NCORES):
        o = res.results[c]["out"]
        for g in range(NG):
            sl = slice(512 * g + 64 * c, 512 * g + 64 * c + 64)
            out[0, sl] = o[128 * g:128 * g + 64]
            out[1, sl] = o[128 * g + 64:128 * (g + 1)]
    if _trace:
        kernel.last_results = res
    return out


# revision 23
# speedup vs baseline: 1.0159x; 1.0159x over previous
"""TRN2 Bass kernel for nn_DecoderLayer_47175920779446.

Full decoder layer: qkv (mul-bias) -> 16-head attention -> +res -> LN ->
FFN(relu, mul-bias) -> +res -> LN, on x[2, 2048, 1024] fp32.

Sharding (8 cores): attention is sharded by (batch, 4 heads): core c handles
batch c//4, heads 4*(c%4)..4*(c%4)+3 over all 2048 tokens of its batch.
Attention output resharding uses FOUR per-query-group AllToAlls (fired as
each group's heads complete, so they overlap stage_b): FFN tokens are
assigned uniformly -- core d owns, for each group g, batch-0 tokens
[512g+64d, +64) and batch-1 tokens [512g+64d, +64). LN1/FFN/LN2 then run
token-sharded (512 tokens per core) with replicated weights, pipelined per
128-token chunk behind the collectives.

Precision: scores need ~fp32 accuracy (std ~256 feeding exp): q,k chain runs
fp32r (11-bit mantissa) projections, then an exact bf16 hi/lo split with a
2-matmul scheme: S = qh*kh + m_hat (main, K=65 with a fused bias row) plus
[qh;ql]*[kl;kh] (cross, K=128). V/P/FFN run bf16; residuals/LN run fp32.
"""
import contextlib
import numpy as np
import ml_dtypes

import concourse.bass as bass
import concourse.tile as tile
from concourse import bacc, mybir
from concourse.bass_utils import run_bass_kernel_spmd
from concourse.bass_interp import get_hw_module
from concourse.masks import make_identity

H, NH, HD, FF = 1024, 16, 64, 4096
B, T = 2, 2048
EPS = 1e-6
NCORES = 8
HPC = NH // 4          # 4 heads per core
TOK = (B * T) // NCORES  # 512 tokens per core
NKC = T // 128         # 16 key chunks
NG = T // 512          # 4 query groups
KCH = H // 128         # 8 contraction chunks for qkv
f32, f32r, bf16 = mybir.dt.float32, mybir.dt.float32r, mybir.dt.bfloat16
AF = mybir.ActivationFunctionType
ALU = mybir.AluOpType


def _round_mant(x, bits=11):
    xi = np.ascontiguousarray(x, np.float32).view(np.int32)
    shift = 23 - bits
    bias = (1 << (shift - 1)) - 1 + ((xi >> shift) & 1)
    xi = (xi + bias) & ~((1 << shift) - 1)
    return xi.view(np.float32)


def _build_program(sim_single=False):
    nc = bacc.Bacc("TRN2", target_bir_lowering=False, debug=False,
                   num_devices=1 if sim_single else NCORES)
    ap = {}
    ap["xT"] = nc.dram_tensor("xT", [H, T], f32r, kind="ExternalInput").ap()
    ap["xres"] = nc.dram_tensor("xres", [TOK, H], f32, kind="ExternalInput").ap()
    for w in ("wq", "wk", "wv"):
        ap[w] = nc.dram_tensor(w, [H, 4 * HD], f32r, kind="ExternalInput").ap()
    ap["w1"] = nc.dram_tensor("w1", [H, FF], bf16, kind="ExternalInput").ap()
    ap["w2"] = nc.dram_tensor("w2", [FF, H], bf16, kind="ExternalInput").ap()
    ap["lnw"] = nc.dram_tensor("lnw", [4, H], f32, kind="ExternalInput").ap()
    out_ap = nc.dram_tensor("out", [TOK, H], f32, kind="ExternalOutput").ap()

    with tile.TileContext(nc) as tc:
        ctx = contextlib.ExitStack()
        with ctx:
            const = ctx.enter_context(tc.tile_pool(name="const", bufs=1))
            dram = ctx.enter_context(tc.tile_pool(name="dram", bufs=1, space="DRAM"))

            ident = const.tile([128, 128], f32)
            make_identity(nc, ident[:])
            identB = const.tile([128, 128], bf16)
            make_identity(nc, identB[:])

            # per-group collective buffers: [dst core, 64 tok, 4 heads * 64]
            a2a_in = [dram.tile([NCORES, 64, 4 * HD], f32, name=f"a2ai{g}")
                      for g in range(NG)]
            a2a_out = [dram.tile([NCORES, 64, 4 * HD], f32, name=f"a2ao{g}")
                       for g in range(NG)]

            # ---------------- attention scope ----------------
            actx = contextlib.ExitStack()
            with actx:
                qk = actx.enter_context(tc.tile_pool(name="qk", bufs=1))
                sb = actx.enter_context(tc.tile_pool(name="sb", bufs=3))
                small = actx.enter_context(tc.tile_pool(name="small", bufs=4))
                psn = actx.enter_context(
                    tc.tile_pool(name="psn", bufs=2, space="PSUM"))
                pss = actx.enter_context(
                    tc.tile_pool(name="pss", bufs=2, space="PSUM"))
                pso = actx.enter_context(
                    tc.tile_pool(name="pso", bufs=1, space="PSUM"))
                psm = actx.enter_context(
                    tc.tile_pool(name="psm", bufs=1, space="PSUM"))

                # per-head score operands
                til_q, til_k, cr_q, cr_k = {}, {}, {}, {}
                for h in range(HPC):
                    til_q[h] = qk.tile([65, T], bf16, name=f"til_q{h}", tag="tq", bufs=HPC)
                    til_k[h] = qk.tile([65, T], bf16, name=f"til_k{h}", tag="tk", bufs=HPC)
                    cr_q[h] = qk.tile([128, T], bf16, name=f"cr_q{h}", tag="cq", bufs=HPC)
                    cr_k[h] = qk.tile([128, T], bf16, name=f"cr_k{h}", tag="ck", bufs=HPC)
                    nc.gpsimd.memset(til_k[h][64:65, :], 1.0)
                vn = []
                for kc in range(NKC):
                    v = qk.tile([128, HPC, 65], bf16, name=f"vn{kc}", tag="vn", bufs=NKC)
                    nc.gpsimd.memset(v[:, :, 64:65], 1.0)
                    vn.append(v)

                # ---- QKV projection (own scope: weights + xT staging free
                # early so the w1 prefetch can reuse the SBUF) ----
                pctx = contextlib.ExitStack()
                with pctx:
                    wpool = pctx.enter_context(tc.tile_pool(name="wpool", bufs=1))
                    xgp = pctx.enter_context(tc.tile_pool(name="xgp", bufs=4))

                    w_sb = {}
                    for w in ("wq", "wk", "wv"):
                        w_sb[w] = wpool.tile([128, KCH, 4 * HD], f32r, name=f"sb_{w}")
                    xgs = [xgp.tile([128, KCH, 512], f32r, name=f"xg{g}",
                                    tag="xg", bufs=4) for g in range(NG)]
                    # chunked loads, K-path first so wk matmuls start ~1us in
                    wrr = {w: ap[w].rearrange("(a p) c -> p a c", p=128)
                           for w in ("wq", "wk", "wv")}
                    xrr = ap["xT"].rearrange("(a p) t -> p a t", p=128)
                    for a in range(KCH):
                        nc.sync.dma_start(w_sb["wk"][:, a, :], wrr["wk"][:, a, :])
                        nc.sync.dma_start(
                            xgs[0][:, a, :], xrr[:, a, 0:512])
                    for w in ("wq", "wv"):
                        for a in range(KCH):
                            nc.sync.dma_start(w_sb[w][:, a, :], wrr[w][:, a, :])
                    for g in range(1, NG):
                        for a in range(KCH):
                            nc.sync.dma_start(
                                xgs[g][:, a, :], xrr[:, a, 512 * g:512 * (g + 1)])

                    def proj_pass(name, til, cr, g):
                        gsl = slice(512 * g, 512 * (g + 1))
                        for hp in range(2):  # head pairs
                            p = pss.tile([128, 512], f32, tag="st", name="pqk")
                            for a in range(KCH):
                                nc.tensor.matmul(
                                    p[:], w_sb[name][:, a, 128 * hp:128 * (hp + 1)],
                                    xgs[g][:, a, :], start=(a == 0), stop=(a == KCH - 1))
                            for hl in range(2):
                                h = 2 * hp + hl
                                rows = slice(64 * hl, 64 * (hl + 1))
                                nc.scalar.activation(til[h][0:64, gsl], p[rows, :], AF.Copy)
                                if name == "wq":
                                    hi_rows, lo_rows = slice(0, 64), slice(64, 128)
                                else:
                                    hi_rows, lo_rows = slice(64, 128), slice(0, 64)
                                nc.sync.dma_start(cr[h][hi_rows, gsl], til[h][0:64, gsl])
                                nc.vector.scalar_tensor_tensor(
                                    out=cr[h][lo_rows, gsl], in0=p[rows, :], scalar=1.0,
                                    in1=til[h][0:64, gsl], op0=ALU.mult, op1=ALU.subtract)

                    for g in range(NG):
                        proj_pass("wk", til_k, cr_k, g)
                        proj_pass("wq", til_q, cr_q, g)
                        for tt in range(4):  # V natural per token tile
                            kc = 4 * g + tt
                            p = pss.tile([128, 4 * HD], f32, tag="st", name="pv")
                            for a in range(KCH):
                                nc.tensor.matmul(
                                    p[:], xgs[g][:, a, 128 * tt:128 * (tt + 1)],
                                    w_sb["wv"][:, a, :], start=(a == 0), stop=(a == KCH - 1))
                            nc.scalar.activation(
                                vn[kc][:, :, 0:64],
                                p[:].rearrange("p (h d) -> p h d", h=HPC), AF.Copy)
                # pctx closed: reserve right-side SBUF for the FFN1 weights
                # (streamed in during attention; the left side stays clear of
                # the attention tiles, so no WAR serialization) and for the
                # LN1-side tiles that run inside the attention loop.
                w1p = ctx.enter_context(
                    tc.tile_pool(name="w1p", bufs=1, side="right"))
                lnp = ctx.enter_context(
                    tc.tile_pool(name="lnp", bufs=1, side="right"))
                w1full = w1p.tile([128, KCH, FF], bf16, name="w1full")
                w1rr = ap["w1"].rearrange("(a p) f -> p a f", p=128)

                def prefetch_chunk(i):
                    # w1 by f-blocks, in FFN1 consumption order
                    fsl = slice(512 * i, 512 * (i + 1))
                    nc.sync.dma_start(w1full[:, :, fsl], w1rr[:, :, fsl])

                lnbc1 = {}
                for i, nm in enumerate(("g1", "b1")):
                    lnbc1[nm] = lnp.tile([128, H], f32, name=f"ln_{nm}",
                                         tag="lnbc1", bufs=2)
                    nc.sync.dma_start(
                        lnbc1[nm][:], ap["lnw"][i, :].partition_broadcast(128))

                def layer_norm_to(dst, src, g_bc, b_bc, work, pool):
                    """dst = gamma*(src-mean)/(std_unbiased+EPS)+beta, [128,H]."""
                    stats = pool.tile([128, 2, 6], f32, tag="stats", name="stats",
                                      bufs=2)
                    for hf in range(2):
                        nc.vector.bn_stats(stats[:, hf, :],
                                           src[:, 512 * hf:512 * (hf + 1)])
                    mv = pool.tile([128, 2], f32, tag="mv", name="mv", bufs=2)
                    nc.vector.bn_aggr(mv[:], stats[:])
                    sd = pool.tile([128, 1], f32, tag="sd", name="sd", bufs=2)
                    nc.scalar.activation(sd[:], mv[:, 1:2], AF.Sqrt,
                                         scale=float(H) / (H - 1))
                    nc.vector.tensor_scalar_add(sd[:], sd[:], EPS)
                    rs = pool.tile([128, 1], f32, tag="rs", name="rs", bufs=2)
                    nc.vector.reciprocal(rs[:], sd[:])
                    nc.vector.tensor_scalar(out=work[:], in0=src[:],
                                            scalar1=mv[:, 0:1], scalar2=rs[:],
                                            op0=ALU.subtract, op1=ALU.mult)
                    nc.vector.tensor_mul(work[:], work[:], g_bc[:])
                    nc.vector.tensor_add(dst[:], work[:], b_bc[:])

                out1c = {}

                def ln1(g):
                    # gather this core's 128 tokens of chunk g: rows 0:64 from
                    # batch-0 sources (0..3), 64:128 from batch-1 (4..7)
                    at = lnp.tile([128, H], f32, tag="ta", name="at", bufs=2)
                    for s in range(4):
                        csl = slice(256 * s, 256 * (s + 1))
                        nc.sync.dma_start(at[0:64, csl], a2a_out[g][s, :, :])
                        nc.sync.dma_start(at[64:128, csl], a2a_out[g][s + 4, :, :])
                    xt = lnp.tile([128, H], f32, tag="tc", name="xt", bufs=1)
                    nc.sync.dma_start(xt[:], ap["xres"][128 * g:128 * (g + 1), :])
                    nc.vector.tensor_add(at[:], at[:], xt[:])
                    wk_ = lnp.tile([128, H], f32, tag="td", name="wk_", bufs=1)
                    o1 = lnp.tile([128, H], bf16, name="out1c", tag="o1", bufs=4)
                    layer_norm_to(o1[:], at, lnbc1["g1"], lnbc1["b1"], wk_, lnp)
                    out1c[g] = o1

                # ---- attention, software-pipelined over (group, head) units
                # g-major so each group's AllToAll fires 1/4 of the way in.
                units = [(h, g) for g in range(NG) for h in range(HPC)]

                def stage_a1(h, g):
                    # natural-S matmuls + DVE max reduces (no PE dependency on DVE)
                    mstage = small.tile([128, 4], bf16, tag="mstage", name="mstage", bufs=2)
                    for qt in range(4):
                        qsl = slice(512 * g + 128 * qt, 512 * g + 128 * (qt + 1))
                        negmax = []
                        for half in range(2):
                            sn = psn.tile([128, 1024], f32, name="sn")
                            for j in range(2):
                                ks = slice(1024 * half + 512 * j,
                                           1024 * half + 512 * (j + 1))
                                nc.tensor.matmul(
                                    sn[:, 512 * j:512 * (j + 1)],
                                    til_q[h][0:64, qsl], til_k[h][0:64, ks],
                                    start=True, stop=True)
                            nm = small.tile([128, 1], bf16, tag="nm", name="nm")
                            nc.vector.tensor_reduce(
                                nm[:], sn[:], axis=mybir.AxisListType.X,
                                op=ALU.max, negate=True)
                            negmax.append(nm)
                        nc.vector.tensor_tensor(
                            mstage[:, qt:qt + 1], negmax[0][:], negmax[1][:], ALU.min)
                    return mstage

                def stage_a2(h, g, mstage):
                    # emitted a period later so the PE transpose never waits on DVE
                    for qt in range(4):
                        qsl = slice(512 * g + 128 * qt, 512 * g + 128 * (qt + 1))
                        mt = psm.tile([1, 128], bf16, tag="mt", name="mt")
                        nc.tensor.transpose(mt[:], mstage[:, qt:qt + 1], identB[:])
                        nc.vector.tensor_copy(til_q[h][64:65, qsl], mt[:])

                def stage_b(h, g):
                    gsl = slice(512 * g, 512 * (g + 1))
                    o_acc = pso.tile([65, 512], f32, name="o_acc")
                    pts = {}
                    PVLAG = 2

                    def pv(kc):
                        nc.tensor.matmul(o_acc[:], vn[kc][:, h, :], pts.pop(kc)[:],
                                         start=(kc == 0), stop=(kc == NKC - 1))

                    for kc in range(NKC):
                        ksl = slice(128 * kc, 128 * (kc + 1))
                        st = pss.tile([128, 512], f32, tag="st", name="st")
                        nc.tensor.matmul(st[:], til_k[h][0:65, ksl],
                                         til_q[h][0:65, gsl], start=True, stop=False)
                        nc.tensor.matmul(st[:], cr_k[h][:, ksl],
                                         cr_q[h][:, gsl], start=False, stop=True)
                        pt = sb.tile([128, 512], bf16, tag="pt", name="pt", bufs=6)
                        nc.scalar.activation(pt[:], st[:], AF.Exp)
                        pts[kc] = pt
                        if kc >= PVLAG:
                            pv(kc - PVLAG)
                    for kc in range(NKC - PVLAG, NKC):
                        pv(kc)
                    ot = sb.tile([65, 512], f32, tag="ot", name="ot", bufs=2)
                    nc.scalar.activation(ot[:], o_acc[:], AF.Copy)
                    # transpose to natural, scale by 1/denom, ship to a2a_in:
                    # rows 0:64 of token tile tt go to dst core 2tt, 64:128 to
                    # 2tt+1 (uniform token ownership, no duplication).
                    for tt in range(4):
                        op_ = psm.tile([128, 65], f32, tag="mt", name="opt")
                        nc.tensor.transpose(
                            op_[:], ot[0:65, 128 * tt:128 * (tt + 1)],
                            ident[0:65, 0:65])
                        rc = small.tile([128, 1], f32, tag="rc", name="rc")
                        nc.vector.reciprocal(rc[:], op_[:, 64:65])
                        ob = sb.tile([128, HD], f32, tag="ob", name="ob", bufs=4)
                        nc.vector.tensor_scalar_mul(ob[:], op_[:, 0:64], rc[:])
                        nc.sync.dma_start(
                            a2a_in[g][2 * tt, :, 64 * h:64 * (h + 1)], ob[0:64, :])
                        nc.sync.dma_start(
                            a2a_in[g][2 * tt + 1, :, 64 * h:64 * (h + 1)], ob[64:128, :])

                def fire_collective(g):
                    if sim_single:
                        # timing stand-in for the 512KB per-group AllToAll
                        nc.sync.dma_start(a2a_out[g][:], a2a_in[g][:])
                    else:
                        nc.gpsimd.collective_compute(
                            "AllToAll", ALU.bypass,
                            replica_groups=[list(range(NCORES))],
                            ins=[a2a_in[g].opt()], outs=[a2a_out[g].opt()])

                LOOKAHEAD = 3
                mstages, done_a2 = {}, set()
                for k in range(min(LOOKAHEAD, len(units))):
                    mstages[k] = stage_a1(*units[k])
                    stage_a2(*units[k], mstages.pop(k))
                    done_a2.add(k)
                for i, (h, g) in enumerate(units):
                    j = i + LOOKAHEAD
                    if j < len(units):
                        mstages[j] = stage_a1(*units[j])
                    j2 = i + LOOKAHEAD - 1
                    if j2 < len(units) and j2 not in done_a2:
                        stage_a2(*units[j2], mstages.pop(j2))
                        done_a2.add(j2)
                    stage_b(h, g)
                    if 1 <= i <= KCH:
                        prefetch_chunk(i - 1)
                    if h == HPC - 1:
                        fire_collective(g)
                        ln1(g)

            # ---------------- FFN / LN tail: pipelined per 128-token chunk --
            # (LN1 for every chunk already ran inside the attention loop.)
            fctx = contextlib.ExitStack()
            with fctx:
                fsb = fctx.enter_context(tc.tile_pool(name="fsb", bufs=1))
                o1p = fctx.enter_context(tc.tile_pool(name="o1p", bufs=1))
                w2p = fctx.enter_context(tc.tile_pool(name="w2p", bufs=1))
                fsm = fctx.enter_context(tc.tile_pool(name="fsm", bufs=2))
                psf = fctx.enter_context(
                    tc.tile_pool(name="psf", bufs=2, space="PSUM"))
                psg = fctx.enter_context(
                    tc.tile_pool(name="psg", bufs=2, space="PSUM"))

                lnbc2 = {}
                for i, nm in enumerate(("g2", "b2")):
                    lnbc2[nm] = o1p.tile([128, H], f32, name=f"ln_{nm}",
                                         tag="lnbc2", bufs=2)
                    nc.sync.dma_start(
                        lnbc2[nm][:], ap["lnw"][2 + i, :].partition_broadcast(128))

                # w2 lands in SBUF freed by the attention tiles; FFN2 consumes
                # ft ascending, so chunk the load in that order.
                w2full = w2p.tile([128, FF // 128, H], bf16, name="w2full")
                w2rr = ap["w2"].rearrange("(a p) o -> p a o", p=128)
                for c in range(8):
                    nc.gpsimd.dma_start(
                        w2full[:, 4 * c:4 * (c + 1), :],
                        w2rr[:, 4 * c:4 * (c + 1), :])

                out1T, htc = {}, {}

                def transp1(g):
                    o1T = o1p.tile([128, KCH, 128], bf16, name="out1T",
                                   tag="o1T", bufs=2)
                    for a in range(KCH):
                        tp = psg.tile([128, 128], bf16, tag="tp", name="tp", bufs=2)
                        nc.tensor.transpose(
                            tp[:], out1c[g][:, 128 * a:128 * (a + 1)], identB[:])
                        nc.scalar.activation(o1T[:, a, :], tp[:], AF.Copy)
                    out1T[g] = o1T

                def ffn1(g):
                    ht = o1p.tile([128, FF // 128, 128], bf16, name="htc",
                                  tag="ht", bufs=2)
                    for fb in range(KCH):  # 8 blocks of 512 ff rows
                        hp_ = psf.tile([128, 512], f32, tag="hp", name="hp", bufs=3)
                        for fq in range(4):
                            for a in range(KCH):
                                nc.tensor.matmul(
                                    hp_[:, 128 * fq:128 * (fq + 1)],
                                    w1full[:, a, 512 * fb + 128 * fq:512 * fb + 128 * (fq + 1)],
                                    out1T[g][:, a, :], start=(a == 0), stop=(a == KCH - 1))
                        nc.scalar.activation(
                            ht[:, 4 * fb:4 * (fb + 1), :],
                            hp_[:].rearrange("p (q t) -> p q t", q=4), AF.Relu)
                    htc[g] = ht

                def ffn2_ln2(g):
                    h2 = fsb.tile([128, H], f32, tag="ta", name="h2", bufs=1)
                    for oc in range(2):
                        acc = psf.tile([128, 512], f32, tag="o2", name="o2acc", bufs=2)
                        for ft in range(FF // 128):
                            nc.tensor.matmul(
                                acc[:], htc[g][:, ft, :],
                                w2full[:, ft, 512 * oc:512 * (oc + 1)],
                                start=(ft == 0), stop=(ft == FF // 128 - 1))
                        nc.vector.tensor_add(
                            h2[:, 512 * oc:512 * (oc + 1)],
                            out1c[g][:, 512 * oc:512 * (oc + 1)], acc[:])
                    fin = fsb.tile([128, H], f32, tag="tb", name="fin", bufs=1)
                    wk2 = fsb.tile([128, H], f32, tag="tc", name="wk2", bufs=1)
                    layer_norm_to(fin, h2, lnbc2["g2"], lnbc2["b2"], wk2, fsm)
                    nc.sync.dma_start(out_ap[128 * g:128 * (g + 1), :], fin[:])

                # run ffn2 two chunks behind ffn1 so its w2 reads land after
                # the (attention-SBUF-gated) w2 load has streamed in
                for g in range(NG):
                    transp1(g)
                    ffn1(g)
                    if g >= 1:
                        ffn2_ln2(g - 1)
                ffn2_ln2(NG - 1)

    nc.compile()
    if not sim_single:
        nc.m = get_hw_module(nc.m)
    return nc


_NC_CACHE = {}


def _get_program():
    if "nc" not in _NC_CACHE:
        _NC_CACHE["nc"] = _build_program()
    return _NC_CACHE["nc"]


def _prep_inputs(x, Wqkv, bqkv, W1, b1, W2, b2, gamma1, beta1, gamma2, beta2):
    """Host-side slicing/folding into per-core in_maps."""
    x = np.asarray(x, np.float32)
    Wqkv = np.asarray(Wqkv, np.float32)
    bqkv = np.asarray(bqkv, np.float32)
    d = np.arange(HD)
    hh = np.arange(NH)
    # qkv reshape in reference: [B,T,HD,3,NH] -> col = d*48 + k*16 + h
    cols = d[:, None, None] * (3 * NH) + np.arange(3)[None, :, None] * NH \
        + hh[None, None, :]
    Wq = Wqkv[:, cols[:, 0, :]] * (bqkv[cols[:, 0, :]] / np.sqrt(H))[None]
    Wk = Wqkv[:, cols[:, 1, :]] * bqkv[cols[:, 1, :]][None]
    Wv = Wqkv[:, cols[:, 2, :]] * bqkv[cols[:, 2, :]][None]
    # -> [H, HD, NH]; per-core head-major layout [H, 4*HD] (head-local major)
    Wq = np.transpose(Wq, (0, 2, 1))  # [H, NH, HD]
    Wk = np.transpose(Wk, (0, 2, 1))
    Wv = np.transpose(Wv, (0, 2, 1))
    W1e = (np.asarray(W1, np.float32) * np.asarray(b1, np.float32)[None]) \
        .astype(ml_dtypes.bfloat16)
    W2e = (np.asarray(W2, np.float32) * np.asarray(b2, np.float32)[None]) \
        .astype(ml_dtypes.bfloat16)
    lnw = np.stack([gamma1, beta1, gamma2, beta2]).astype(np.float32)
    xT = [_round_mant(np.ascontiguousarray(x[b].T)) for b in range(B)]
    in_maps = []
    for c in range(NCORES):
        b, grp = c // 4, c % 4
        heads = slice(4 * grp, 4 * grp + 4)
        # xres rows for core c: per group g, 64 batch-0 then 64 batch-1 tokens
        xres = np.concatenate([
            np.concatenate([x[0, 512 * g + 64 * c:512 * g + 64 * c + 64, :],
                            x[1, 512 * g + 64 * c:512 * g + 64 * c + 64, :]])
            for g in range(NG)])
        in_maps.append({
            "xT": xT[b],
            "xres": np.ascontiguousarray(xres),
            "wq": _round_mant(Wq[:, heads, :].reshape(H, 4 * HD)),
            "wk": _round_mant(Wk[:, heads, :].reshape(H, 4 * HD)),
            "wv": _round_mant(Wv[:, heads, :].reshape(H, 4 * HD)),
            "w1": W1e, "w2": W2e, "lnw": lnw,
        })
    return in_maps


def kernel(x, Wqkv, bqkv, W1, b1, W2, b2, gamma1, beta1, gamma2, beta2,
           _trace=False):
    nc = _get_program()
    in_maps = _prep_inputs(x, Wqkv, bqkv, W1, b1, W2, b2,
                           gamma1, beta1, gamma2, beta2)
    res = run_bass_kernel_spmd(nc, in_maps, core_ids=list(range(NCORES)),
                               trace=_trace)
    out = np.empty((B, T, H), np.float32)
    for c in range(NCORES):
        o = res.results[c]["out"]
        for g in range(NG):
            sl = slice(512 * g + 64 * c, 512 * g + 64 * c + 64)
            out[0, sl] = o[128 * g:128 * g + 64]
            out[1, sl] = o[128 * g + 64:128 * (g + 1)]
    if _trace:
        kernel.last_results = res
    return out


# revision 26
# speedup vs baseline: 1.0542x; 1.0377x over previous
"""TRN2 Bass kernel for nn_DecoderLayer_47175920779446.

Full decoder layer: qkv (mul-bias) -> 16-head attention -> +res -> LN ->
FFN(relu, mul-bias) -> +res -> LN, on x[2, 2048, 1024] fp32.

Sharding (8 cores): attention is sharded by (batch, 4 heads): core c handles
batch c//4, heads 4*(c%4)..4*(c%4)+3 over all 2048 tokens of its batch.
Attention output resharding uses FOUR per-query-group AllToAlls (fired as
each group's heads complete, so they overlap stage_b): FFN tokens are
assigned uniformly -- core d owns, for each group g, batch-0 tokens
[512g+64d, +64) and batch-1 tokens [512g+64d, +64). LN1/FFN/LN2 then run
token-sharded (512 tokens per core) with replicated weights, pipelined per
128-token chunk behind the collectives.

Precision: scores need ~fp32 accuracy (std ~256 feeding exp): q,k chain runs
fp32r (11-bit mantissa) projections, then an exact bf16 hi/lo split with a
2-matmul scheme: S = qh*kh + m_hat (main, K=65 with a fused bias row) plus
[qh;ql]*[kl;kh] (cross, K=128). V/P/FFN run bf16; residuals/LN run fp32.
"""
import contextlib
import numpy as np
import ml_dtypes

import concourse.bass as bass
import concourse.tile as tile
from concourse import bacc, mybir
from concourse.bass_utils import run_bass_kernel_spmd
from concourse.bass_interp import get_hw_module
from concourse.masks import make_identity

H, NH, HD, FF = 1024, 16, 64, 4096
B, T = 2, 2048
EPS = 1e-6
NCORES = 8
HPC = NH // 4          # 4 heads per core
TOK = (B * T) // NCORES  # 512 tokens per core
NKC = T // 128         # 16 key chunks
NG = T // 512          # 4 query groups
KCH = H // 128         # 8 contraction chunks for qkv
f32, f32r, bf16 = mybir.dt.float32, mybir.dt.float32r, mybir.dt.bfloat16
AF = mybir.ActivationFunctionType
ALU = mybir.AluOpType


def _round_mant(x, bits=11):
    xi = np.ascontiguousarray(x, np.float32).view(np.int32)
    shift = 23 - bits
    bias = (1 << (shift - 1)) - 1 + ((xi >> shift) & 1)
    xi = (xi + bias) & ~((1 << shift) - 1)
    return xi.view(np.float32)


def _build_program(sim_single=False):
    nc = bacc.Bacc("TRN2", target_bir_lowering=False, debug=False,
                   num_devices=1 if sim_single else NCORES)
    ap = {}
    ap["xT"] = nc.dram_tensor("xT", [H, T], f32r, kind="ExternalInput").ap()
    ap["xres"] = nc.dram_tensor("xres", [TOK, H], f32, kind="ExternalInput").ap()
    for w in ("wq", "wk", "wv"):
        ap[w] = nc.dram_tensor(w, [H, 4 * HD], f32r, kind="ExternalInput").ap()
    ap["w1"] = nc.dram_tensor("w1", [H, FF], bf16, kind="ExternalInput").ap()
    ap["w2"] = nc.dram_tensor("w2", [FF, H], bf16, kind="ExternalInput").ap()
    ap["lnw"] = nc.dram_tensor("lnw", [4, H], f32, kind="ExternalInput").ap()
    out_ap = nc.dram_tensor("out", [TOK, H], f32, kind="ExternalOutput").ap()

    with tile.TileContext(nc) as tc:
        ctx = contextlib.ExitStack()
        with ctx:
            const = ctx.enter_context(tc.tile_pool(name="const", bufs=1))
            dram = ctx.enter_context(tc.tile_pool(name="dram", bufs=1, space="DRAM"))

            ident = const.tile([128, 128], f32)
            make_identity(nc, ident[:])
            identB = const.tile([128, 128], bf16)
            make_identity(nc, identB[:])

            # per-group collective buffers: [dst core, 64 tok, 4 heads * 64]
            a2a_in = [dram.tile([NCORES, 64, 4 * HD], f32, name=f"a2ai{g}")
                      for g in range(NG)]
            a2a_out = [dram.tile([NCORES, 64, 4 * HD], f32, name=f"a2ao{g}")
                       for g in range(NG)]

            # ---------------- attention scope ----------------
            actx = contextlib.ExitStack()
            with actx:
                qk = actx.enter_context(tc.tile_pool(name="qk", bufs=1))
                sb = actx.enter_context(tc.tile_pool(name="sb", bufs=3))
                small = actx.enter_context(tc.tile_pool(name="small", bufs=4))
                psn = actx.enter_context(
                    tc.tile_pool(name="psn", bufs=2, space="PSUM"))
                pss = actx.enter_context(
                    tc.tile_pool(name="pss", bufs=3, space="PSUM"))
                pso = actx.enter_context(
                    tc.tile_pool(name="pso", bufs=2, space="PSUM"))
                psm = actx.enter_context(
                    tc.tile_pool(name="psm", bufs=1, space="PSUM"))

                # per-head score operands
                til_q, til_k, cr_q, cr_k = {}, {}, {}, {}
                for h in range(HPC):
                    til_q[h] = qk.tile([65, T], bf16, name=f"til_q{h}", tag="tq", bufs=HPC)
                    til_k[h] = qk.tile([65, T], bf16, name=f"til_k{h}", tag="tk", bufs=HPC)
                    cr_q[h] = qk.tile([128, T], bf16, name=f"cr_q{h}", tag="cq", bufs=HPC)
                    cr_k[h] = qk.tile([128, T], bf16, name=f"cr_k{h}", tag="ck", bufs=HPC)
                    nc.gpsimd.memset(til_k[h][64:65, :], 1.0)
                vn = []
                for kc in range(NKC):
                    v = qk.tile([128, HPC, 65], bf16, name=f"vn{kc}", tag="vn", bufs=NKC)
                    nc.gpsimd.memset(v[:, :, 64:65], 1.0)
                    vn.append(v)

                # ---- QKV projection (own scope: weights + xT staging free
                # early so the w1 prefetch can reuse the SBUF) ----
                pctx = contextlib.ExitStack()
                with pctx:
                    wpool = pctx.enter_context(tc.tile_pool(name="wpool", bufs=1))
                    xgp = pctx.enter_context(tc.tile_pool(name="xgp", bufs=4))

                    w_sb = {}
                    for w in ("wq", "wk", "wv"):
                        w_sb[w] = wpool.tile([128, KCH, 4 * HD], f32r, name=f"sb_{w}")
                    xgs = [xgp.tile([128, KCH, 512], f32r, name=f"xg{g}",
                                    tag="xg", bufs=4) for g in range(NG)]
                    # chunked loads, K-path first so wk matmuls start ~1us in
                    wrr = {w: ap[w].rearrange("(a p) c -> p a c", p=128)
                           for w in ("wq", "wk", "wv")}
                    xrr = ap["xT"].rearrange("(a p) t -> p a t", p=128)
                    for a in range(KCH):
                        nc.sync.dma_start(w_sb["wk"][:, a, :], wrr["wk"][:, a, :])
                        nc.sync.dma_start(
                            xgs[0][:, a, :], xrr[:, a, 0:512])
                    for w in ("wq", "wv"):
                        for a in range(KCH):
                            nc.sync.dma_start(w_sb[w][:, a, :], wrr[w][:, a, :])
                    for g in range(1, NG):
                        for a in range(KCH):
                            nc.sync.dma_start(
                                xgs[g][:, a, :], xrr[:, a, 512 * g:512 * (g + 1)])

                    def proj_pass(name, til, cr, g):
                        gsl = slice(512 * g, 512 * (g + 1))
                        for hp in range(2):  # head pairs
                            p = pss.tile([128, 512], f32, tag="st", name="pqk")
                            for a in range(KCH):
                                nc.tensor.matmul(
                                    p[:], w_sb[name][:, a, 128 * hp:128 * (hp + 1)],
                                    xgs[g][:, a, :], start=(a == 0), stop=(a == KCH - 1))
                            for hl in range(2):
                                h = 2 * hp + hl
                                rows = slice(64 * hl, 64 * (hl + 1))
                                nc.scalar.activation(til[h][0:64, gsl], p[rows, :], AF.Copy)
                                if name == "wq":
                                    hi_rows, lo_rows = slice(0, 64), slice(64, 128)
                                else:
                                    hi_rows, lo_rows = slice(64, 128), slice(0, 64)
                                nc.sync.dma_start(cr[h][hi_rows, gsl], til[h][0:64, gsl])
                                nc.vector.scalar_tensor_tensor(
                                    out=cr[h][lo_rows, gsl], in0=p[rows, :], scalar=1.0,
                                    in1=til[h][0:64, gsl], op0=ALU.mult, op1=ALU.subtract)

                    for g in range(NG):
                        proj_pass("wk", til_k, cr_k, g)
                        proj_pass("wq", til_q, cr_q, g)
                        for tt in range(4):  # V natural per token tile
                            kc = 4 * g + tt
                            p = pss.tile([128, 4 * HD], f32, tag="st", name="pv")
                            for a in range(KCH):
                                nc.tensor.matmul(
                                    p[:], xgs[g][:, a, 128 * tt:128 * (tt + 1)],
                                    w_sb["wv"][:, a, :], start=(a == 0), stop=(a == KCH - 1))
                            nc.scalar.activation(
                                vn[kc][:, :, 0:64],
                                p[:].rearrange("p (h d) -> p h d", h=HPC), AF.Copy)
                # pctx closed: reserve right-side SBUF for the FFN1 weights
                # (streamed in during attention; the left side stays clear of
                # the attention tiles, so no WAR serialization) and for the
                # LN1-side tiles that run inside the attention loop.
                w1p = ctx.enter_context(
                    tc.tile_pool(name="w1p", bufs=1, side="right"))
                lnp = ctx.enter_context(
                    tc.tile_pool(name="lnp", bufs=1, side="right"))
                w1full = w1p.tile([128, KCH, FF], bf16, name="w1full")
                w1rr = ap["w1"].rearrange("(a p) f -> p a f", p=128)

                def prefetch_chunk(i):
                    # w1 by f-blocks, in FFN1 consumption order
                    fsl = slice(512 * i, 512 * (i + 1))
                    nc.sync.dma_start(w1full[:, :, fsl], w1rr[:, :, fsl])

                lnbc1 = {}
                for i, nm in enumerate(("g1", "b1")):
                    lnbc1[nm] = lnp.tile([128, H], f32, name=f"ln_{nm}",
                                         tag="lnbc1", bufs=2)
                    nc.sync.dma_start(
                        lnbc1[nm][:], ap["lnw"][i, :].partition_broadcast(128))

                def layer_norm_to(dst, src, g_bc, b_bc, work, pool):
                    """dst = gamma*(src-mean)/(std_unbiased+EPS)+beta, [128,H]."""
                    stats = pool.tile([128, 2, 6], f32, tag="stats", name="stats",
                                      bufs=2)
                    for hf in range(2):
                        nc.vector.bn_stats(stats[:, hf, :],
                                           src[:, 512 * hf:512 * (hf + 1)])
                    mv = pool.tile([128, 2], f32, tag="mv", name="mv", bufs=2)
                    nc.vector.bn_aggr(mv[:], stats[:])
                    sd = pool.tile([128, 1], f32, tag="sd", name="sd", bufs=2)
                    nc.scalar.activation(sd[:], mv[:, 1:2], AF.Sqrt,
                                         scale=float(H) / (H - 1))
                    nc.vector.tensor_scalar_add(sd[:], sd[:], EPS)
                    rs = pool.tile([128, 1], f32, tag="rs", name="rs", bufs=2)
                    nc.vector.reciprocal(rs[:], sd[:])
                    nc.vector.tensor_scalar(out=work[:], in0=src[:],
                                            scalar1=mv[:, 0:1], scalar2=rs[:],
                                            op0=ALU.subtract, op1=ALU.mult)
                    nc.vector.tensor_mul(work[:], work[:], g_bc[:])
                    nc.vector.tensor_add(dst[:], work[:], b_bc[:])

                out1c = {}

                def ln1(g):
                    # gather this core's 128 tokens of chunk g: rows 0:64 from
                    # batch-0 sources (0..3), 64:128 from batch-1 (4..7)
                    at = lnp.tile([128, H], f32, tag="ta", name="at", bufs=2)
                    for s in range(4):
                        csl = slice(256 * s, 256 * (s + 1))
                        nc.sync.dma_start(at[0:64, csl], a2a_out[g][s, :, :])
                        nc.sync.dma_start(at[64:128, csl], a2a_out[g][s + 4, :, :])
                    xt = lnp.tile([128, H], f32, tag="tc", name="xt", bufs=1)
                    nc.sync.dma_start(xt[:], ap["xres"][128 * g:128 * (g + 1), :])
                    nc.vector.tensor_add(at[:], at[:], xt[:])
                    wk_ = lnp.tile([128, H], f32, tag="td", name="wk_", bufs=1)
                    o1 = lnp.tile([128, H], bf16, name="out1c", tag="o1", bufs=4)
                    layer_norm_to(o1[:], at, lnbc1["g1"], lnbc1["b1"], wk_, lnp)
                    out1c[g] = o1

                # ---- attention, software-pipelined over (group, head) units
                # g-major so each group's AllToAll fires 1/4 of the way in.
                units = [(h, g) for g in range(NG) for h in range(HPC)]

                def stage_a1(h, g):
                    # natural-S matmuls + DVE max reduces (no PE dependency on DVE)
                    mstage = small.tile([128, 4], bf16, tag="mstage", name="mstage", bufs=2)
                    for qt in range(4):
                        qsl = slice(512 * g + 128 * qt, 512 * g + 128 * (qt + 1))
                        negmax = []
                        for j in range(4):
                            sn = psn.tile([128, 512], f32, name="sn")
                            ks = slice(512 * j, 512 * (j + 1))
                            nc.tensor.matmul(
                                sn[:], til_q[h][0:64, qsl], til_k[h][0:64, ks],
                                start=True, stop=True)
                            nm = small.tile([128, 1], bf16, tag="nm", name="nm", bufs=8)
                            nc.vector.tensor_reduce(
                                nm[:], sn[:], axis=mybir.AxisListType.X,
                                op=ALU.max, negate=True)
                            negmax.append(nm)
                        m01 = small.tile([128, 1], bf16, tag="nm", name="m01", bufs=8)
                        m23 = small.tile([128, 1], bf16, tag="nm", name="m23", bufs=8)
                        nc.vector.tensor_tensor(m01[:], negmax[0][:], negmax[1][:], ALU.min)
                        nc.vector.tensor_tensor(m23[:], negmax[2][:], negmax[3][:], ALU.min)
                        nc.vector.tensor_tensor(
                            mstage[:, qt:qt + 1], m01[:], m23[:], ALU.min)
                    return mstage

                def stage_a2(h, g, mstage):
                    # emitted a period later so the PE transpose never waits on DVE
                    for qt in range(4):
                        qsl = slice(512 * g + 128 * qt, 512 * g + 128 * (qt + 1))
                        mt = psm.tile([1, 128], bf16, tag="mt", name="mt")
                        nc.tensor.transpose(mt[:], mstage[:, qt:qt + 1], identB[:])
                        nc.vector.tensor_copy(til_q[h][64:65, qsl], mt[:])

                def stage_b(h, g):
                    gsl = slice(512 * g, 512 * (g + 1))
                    o_acc = pso.tile([65, 512], f32, name="o_acc")
                    pts = {}
                    PVLAG = 2

                    def pv(kc):
                        nc.tensor.matmul(o_acc[:], vn[kc][:, h, :], pts.pop(kc)[:],
                                         start=(kc == 0), stop=(kc == NKC - 1))

                    for kc in range(NKC):
                        ksl = slice(128 * kc, 128 * (kc + 1))
                        st = pss.tile([128, 512], f32, tag="st", name="st")
                        nc.tensor.matmul(st[:], til_k[h][0:65, ksl],
                                         til_q[h][0:65, gsl], start=True, stop=False)
                        nc.tensor.matmul(st[:], cr_k[h][:, ksl],
                                         cr_q[h][:, gsl], start=False, stop=True)
                        pt = sb.tile([128, 512], bf16, tag="pt", name="pt", bufs=6)
                        nc.scalar.activation(pt[:], st[:], AF.Exp)
                        pts[kc] = pt
                        if kc >= PVLAG:
                            pv(kc - PVLAG)
                    for kc in range(NKC - PVLAG, NKC):
                        pv(kc)
                    ot = sb.tile([65, 512], f32, tag="ot", name="ot", bufs=2)
                    nc.scalar.activation(ot[:], o_acc[:], AF.Copy)
                    # transpose to natural, scale by 1/denom, ship to a2a_in:
                    # rows 0:64 of token tile tt go to dst core 2tt, 64:128 to
                    # 2tt+1 (uniform token ownership, no duplication).
                    for tt in range(4):
                        op_ = psm.tile([128, 65], f32, tag="mt", name="opt")
                        nc.tensor.transpose(
                            op_[:], ot[0:65, 128 * tt:128 * (tt + 1)],
                            ident[0:65, 0:65])
                        rc = small.tile([128, 1], f32, tag="rc", name="rc")
                        nc.vector.reciprocal(rc[:], op_[:, 64:65])
                        ob = sb.tile([128, HD], f32, tag="ob", name="ob", bufs=4)
                        nc.vector.tensor_scalar_mul(ob[:], op_[:, 0:64], rc[:])
                        nc.sync.dma_start(
                            a2a_in[g][2 * tt, :, 64 * h:64 * (h + 1)], ob[0:64, :])
                        nc.sync.dma_start(
                            a2a_in[g][2 * tt + 1, :, 64 * h:64 * (h + 1)], ob[64:128, :])

                def fire_collective(g):
                    if sim_single:
                        # timing stand-in for the 512KB per-group AllToAll
                        nc.sync.dma_start(a2a_out[g][:], a2a_in[g][:])
                    else:
                        nc.gpsimd.collective_compute(
                            "AllToAll", ALU.bypass,
                            replica_groups=[list(range(NCORES))],
                            ins=[a2a_in[g].opt()], outs=[a2a_out[g].opt()])

                LOOKAHEAD = 3
                mstages, done_a2 = {}, set()
                for k in range(min(LOOKAHEAD, len(units))):
                    mstages[k] = stage_a1(*units[k])
                    stage_a2(*units[k], mstages.pop(k))
                    done_a2.add(k)
                for i, (h, g) in enumerate(units):
                    j = i + LOOKAHEAD
                    if j < len(units):
                        mstages[j] = stage_a1(*units[j])
                    j2 = i + LOOKAHEAD - 1
                    if j2 < len(units) and j2 not in done_a2:
                        stage_a2(*units[j2], mstages.pop(j2))
                        done_a2.add(j2)
                    stage_b(h, g)
                    if 1 <= i <= KCH:
                        prefetch_chunk(i - 1)
                    if h == HPC - 1:
                        fire_collective(g)
                        ln1(g)

            # ---------------- FFN / LN tail: pipelined per 128-token chunk --
            # (LN1 for every chunk already ran inside the attention loop.)
            fctx = contextlib.ExitStack()
            with fctx:
                fsb = fctx.enter_context(tc.tile_pool(name="fsb", bufs=1))
                o1p = fctx.enter_context(tc.tile_pool(name="o1p", bufs=1))
                w2p = fctx.enter_context(tc.tile_pool(name="w2p", bufs=1))
                fsm = fctx.enter_context(tc.tile_pool(name="fsm", bufs=2))
                psf = fctx.enter_context(
                    tc.tile_pool(name="psf", bufs=2, space="PSUM"))
                psg = fctx.enter_context(
                    tc.tile_pool(name="psg", bufs=2, space="PSUM"))

                lnbc2 = {}
                for i, nm in enumerate(("g2", "b2")):
                    lnbc2[nm] = o1p.tile([128, H], f32, name=f"ln_{nm}",
                                         tag="lnbc2", bufs=2)
                    nc.sync.dma_start(
                        lnbc2[nm][:], ap["lnw"][2 + i, :].partition_broadcast(128))

                # w2 lands in SBUF freed by the attention tiles; FFN2 consumes
                # ft ascending, so chunk the load in that order.
                w2full = w2p.tile([128, FF // 128, H], bf16, name="w2full")
                w2rr = ap["w2"].rearrange("(a p) o -> p a o", p=128)
                for c in range(8):
                    nc.gpsimd.dma_start(
                        w2full[:, 4 * c:4 * (c + 1), :],
                        w2rr[:, 4 * c:4 * (c + 1), :])

                out1T, htc = {}, {}

                def transp1(g):
                    o1T = o1p.tile([128, KCH, 128], bf16, name="out1T",
                                   tag="o1T", bufs=2)
                    for a in range(KCH):
                        tp = psg.tile([128, 128], bf16, tag="tp", name="tp", bufs=2)
                        nc.tensor.transpose(
                            tp[:], out1c[g][:, 128 * a:128 * (a + 1)], identB[:])
                        nc.scalar.activation(o1T[:, a, :], tp[:], AF.Copy)
                    out1T[g] = o1T

                def ffn1(g):
                    ht = o1p.tile([128, FF // 128, 128], bf16, name="htc",
                                  tag="ht", bufs=2)
                    for fb in range(KCH):  # 8 blocks of 512 ff rows
                        hp_ = psf.tile([128, 512], f32, tag="hp", name="hp", bufs=3)
                        for fq in range(4):
                            for a in range(KCH):
                                nc.tensor.matmul(
                                    hp_[:, 128 * fq:128 * (fq + 1)],
                                    w1full[:, a, 512 * fb + 128 * fq:512 * fb + 128 * (fq + 1)],
                                    out1T[g][:, a, :], start=(a == 0), stop=(a == KCH - 1))
                        nc.scalar.activation(
                            ht[:, 4 * fb:4 * (fb + 1), :],
                            hp_[:].rearrange("p (q t) -> p q t", q=4), AF.Relu)
                    htc[g] = ht

                def ffn2_ln2(g):
                    # LN2 stats interleave with the second matmul half, and the
                    # output ships per 512-col half so the final DMA overlaps
                    # the remaining DVE work.
                    h2 = fsb.tile([128, H], f32, tag="ta", name="h2", bufs=1)
                    stats = fsm.tile([128, 2, 6], f32, tag="stats", name="stats",
                                     bufs=2)
                    for oc in range(2):
                        acc = psf.tile([128, 512], f32, tag="o2", name="o2acc", bufs=2)
                        for ft in range(FF // 128):
                            nc.tensor.matmul(
                                acc[:], htc[g][:, ft, :],
                                w2full[:, ft, 512 * oc:512 * (oc + 1)],
                                start=(ft == 0), stop=(ft == FF // 128 - 1))
                        nc.vector.tensor_add(
                            h2[:, 512 * oc:512 * (oc + 1)],
                            out1c[g][:, 512 * oc:512 * (oc + 1)], acc[:])
                        nc.vector.bn_stats(stats[:, oc, :],
                                           h2[:, 512 * oc:512 * (oc + 1)])
                    mv = fsm.tile([128, 2], f32, tag="mv", name="mv", bufs=2)
                    nc.vector.bn_aggr(mv[:], stats[:])
                    sd = fsm.tile([128, 1], f32, tag="sd", name="sd", bufs=2)
                    nc.scalar.activation(sd[:], mv[:, 1:2], AF.Sqrt,
                                         scale=float(H) / (H - 1))
                    nc.vector.tensor_scalar_add(sd[:], sd[:], EPS)
                    rs = fsm.tile([128, 1], f32, tag="rs", name="rs", bufs=2)
                    nc.vector.reciprocal(rs[:], sd[:])
                    fin = fsb.tile([128, H], f32, tag="tb", name="fin", bufs=1)
                    wk2 = fsb.tile([128, H], f32, tag="tc", name="wk2", bufs=1)
                    for oc in range(2):
                        osl = slice(512 * oc, 512 * (oc + 1))
                        nc.vector.tensor_scalar(out=wk2[:, osl], in0=h2[:, osl],
                                                scalar1=mv[:, 0:1], scalar2=rs[:],
                                                op0=ALU.subtract, op1=ALU.mult)
                        nc.vector.tensor_mul(wk2[:, osl], wk2[:, osl],
                                             lnbc2["g2"][:, osl])
                        nc.vector.tensor_add(fin[:, osl], wk2[:, osl],
                                             lnbc2["b2"][:, osl])
                        nc.sync.dma_start(
                            out_ap[128 * g:128 * (g + 1), osl], fin[:, osl])

                # run ffn2 two chunks behind ffn1 so its w2 reads land after
                # the (attention-SBUF-gated) w2 load has streamed in
                for g in range(NG):
                    transp1(g)
                    ffn1(g)
                    if g >= 1:
                        ffn2_ln2(g - 1)
                ffn2_ln2(NG - 1)

    nc.compile()
    if not sim_single:
        nc.m = get_hw_module(nc.m)
    return nc


_NC_CACHE = {}


def _get_program():
    if "nc" not in _NC_CACHE:
        _NC_CACHE["nc"] = _build_program()
    return _NC_CACHE["nc"]


def _prep_inputs(x, Wqkv, bqkv, W1, b1, W2, b2, gamma1, beta1, gamma2, beta2):
    """Host-side slicing/folding into per-core in_maps."""
    x = np.asarray(x, np.float32)
    Wqkv = np.asarray(Wqkv, np.float32)
    bqkv = np.asarray(bqkv, np.float32)
    d = np.arange(HD)
    hh = np.arange(NH)
    # qkv reshape in reference: [B,T,HD,3,NH] -> col = d*48 + k*16 + h
    cols = d[:, None, None] * (3 * NH) + np.arange(3)[None, :, None] * NH \
        + hh[None, None, :]
    Wq = Wqkv[:, cols[:, 0, :]] * (bqkv[cols[:, 0, :]] / np.sqrt(H))[None]
    Wk = Wqkv[:, cols[:, 1, :]] * bqkv[cols[:, 1, :]][None]
    Wv = Wqkv[:, cols[:, 2, :]] * bqkv[cols[:, 2, :]][None]
    # -> [H, HD, NH]; per-core head-major layout [H, 4*HD] (head-local major)
    Wq = np.transpose(Wq, (0, 2, 1))  # [H, NH, HD]
    Wk = np.transpose(Wk, (0, 2, 1))
    Wv = np.transpose(Wv, (0, 2, 1))
    W1e = (np.asarray(W1, np.float32) * np.asarray(b1, np.float32)[None]) \
        .astype(ml_dtypes.bfloat16)
    W2e = (np.asarray(W2, np.float32) * np.asarray(b2, np.float32)[None]) \
        .astype(ml_dtypes.bfloat16)
    lnw = np.stack([gamma1, beta1, gamma2, beta2]).astype(np.float32)
    xT = [_round_mant(np.ascontiguousarray(x[b].T)) for b in range(B)]
    in_maps = []
    for c in range(NCORES):
        b, grp = c // 4, c % 4
        heads = slice(4 * grp, 4 * grp + 4)
        # xres rows for core c: per group g, 64 batch-0 then 64 batch-1 tokens
        xres = np.concatenate([
            np.concatenate([x[0, 512 * g + 64 * c:512 * g + 64 * c + 64, :],
                            x[1, 512 * g + 64 * c:512 * g + 64 * c + 64, :]])
            for g in range(NG)])
        in_maps.append({
            "xT": xT[b],
            "xres": np.ascontiguousarray(xres),
            "wq": _round_mant(Wq[:, heads, :].reshape(H, 4 * HD)),
            "wk": _round_mant(Wk[:, heads, :].reshape(H, 4 * HD)),
            "wv": _round_mant(Wv[:, heads, :].reshape(H, 4 * HD)),
            "w1": W1e, "w2": W2e, "lnw": lnw,
        })
    return in_maps


def kernel(x, Wqkv, bqkv, W1, b1, W2, b2, gamma1, beta1, gamma2, beta2,
           _trace=False):
    nc = _get_program()
    in_maps = _prep_inputs(x, Wqkv, bqkv, W1, b1, W2, b2,
                           gamma1, beta1, gamma2, beta2)
    res = run_bass_kernel_spmd(nc, in_maps, core_ids=list(range(NCORES)),
                               trace=_trace)
    out = np.empty((B, T, H), np.float32)
    for c in range(NCORES):
        o = res.results[c]["out"]
        for g in range(NG):
            sl = slice(512 * g + 64 * c, 512 * g + 64 * c + 64)
            out[0, sl] = o[128 * g:128 * g + 64]
            out[1, sl] = o[128 * g + 64:128 * (g + 1)]
    if _trace:
        kernel.last_results = res
    return out
